# revision 51
# baseline (speedup 1.0000x reference)
"""Trainium2 Bass kernel for nn_PixtralHFVisionModel (8-core TP).

Strategy (Megatron tensor-parallel over 8 NeuronCores, one chip):
  - Patch-embed conv done as matmul (host im2col), replicated on all cores.
  - Activations live TRANSPOSED in SBUF: [hidden(partitions, 8x128), S(free)].
  - Per-core: 2 of 16 heads (q/k/v/o column/row split), 512 of 4096 MLP
    intermediate channels. Partial o-proj / down-proj outputs are
    AllReduced (bf16) across the 8 cores; each core keeps the full f32
    residual stream.
  - Attention is computed per image block (1024 / 512 tokens) -> the
    block-diagonal mask is free. Scores are built transposed [kv, q], so
    softmax-exp output feeds the AV matmul directly (no P transpose);
    denominator comes from an appended ones-column in the V operand.
  - exp without max subtraction (scores*scale is provably small here).
  - RoPE rotate-half via a constant permutation matmul on the PE.
  - Softmax denominator: reciprocal row -> K=1 ones-matmul broadcast.
  - o-proj: both heads stacked on partitions -> single K=128 matmul.
  - Layer loop software-pipelined: each image's MLP AllReduce of layer l
    is consumed at the top of layer l+1, so the collective flies while
    the PE runs the other image / next-layer attention. Image 1 (small)
    goes first each phase so its collective hides under image 0 compute.
  - All collectives are chunked per 512-token block and pipelined:
    each chunk's AllReduce fires as soon as its o-proj / down-proj
    lands, and its residual-add + RMS stats run while the next chunk
    is still in flight.
  - Queue discipline (critical for overlap): the gpsimd queue carries
    ONLY collective_compute (a collective blocks its issuing queue
    until completion); AR-input writes ride the scalar queue right
    behind the PSUM evacuations; AR-output reads + weight loads ride
    the sync queue; norm weights are folded into the projection
    weights on the host.
All matmuls bf16 inputs, f32 PSUM accumulation.
"""
import sys

if "/opt/trn_rl_repo" not in sys.path:
    sys.path.insert(0, "/opt/trn_rl_repo")

import numpy as np
import ml_dtypes

BF16 = ml_dtypes.bfloat16
NCORES = 8
HID = 1024
HD = 64
INTER = 4096
NLAYERS = 4
PATCH = 16
MAXSIDE = 64
THETA = 10000.0
EPS = 1e-5
SCALE = HD ** -0.5
GRIDS = [(32, 32), (32, 16)]
S0, S1 = 1024, 512
S = S0 + S1
CH = 512            # free-dim matmul chunk (one PSUM bank of f32)
NCH = S // CH       # 3
KT = HID // 128     # 8 hidden k-tiles
PKT = 768 // 128    # 6 patch k-tiles
MT_I = 512 // 128   # 4 intermediate m-tiles per core

_CACHE = {}


def _build_nc():
    import concourse.bacc as bacc
    from concourse import tile
    import concourse.mybir as mybir

    dt = mybir.dt
    f32, bf16 = dt.float32, dt.bfloat16
    AF = mybir.ActivationFunctionType
    ALU = mybir.AluOpType

    nc = bacc.Bacc("TRN2", target_bir_lowering=False, debug=False,
                   num_devices=NCORES)

    def din(name, shape, dtype=bf16):
        return nc.dram_tensor(name, shape, dtype, kind="ExternalInput")

    patchesT_d = din("patchesT", [128, PKT, S])
    convWT_d = din("convWT", [128, PKT, HID])
    cos2_d = din("cos2", [128, S])
    sin2_d = din("sin2", [128, S])
    rotP_d = din("rotP", [128, 128])
    lnw_d = din("lnw", [128, KT], f32)
    wq_d = din("wq", [NLAYERS, 128, KT, 128])
    wk_d = din("wk", [NLAYERS, 128, KT, 128])
    wv_d = din("wv", [NLAYERS, 128, KT, 128])
    wo_d = din("wo", [NLAYERS, 128, KT, 128])
    wg_d = din("wg", [NLAYERS, 128, KT, 512])
    wu_d = din("wu", [NLAYERS, 128, KT, 512])
    wd_d = din("wd", [NLAYERS, 128, MT_I, HID])
    out_d = nc.dram_tensor("out", [128, KT, S], f32, kind="ExternalOutput")

    import concourse.bass as bass_mod

    with tile.TileContext(nc) as tc:
        with (
            tc.tile_pool(name="const", bufs=1) as constp,
            tc.tile_pool(name="big", bufs=1) as bigp,
            tc.tile_pool(name="wat", bufs=2) as watp,
            tc.tile_pool(name="wmlp", bufs=2) as wmlpp,
            tc.tile_pool(name="wdp", bufs=1) as wdp,
            tc.tile_pool(name="att", bufs=1) as attp,
            tc.tile_pool(name="wrk1", bufs=1) as wrk1p,
            tc.tile_pool(name="wrk2", bufs=2) as wrk2p,
            tc.tile_pool(name="cast", bufs=2) as castp,
            tc.tile_pool(name="dram", bufs=2, space="DRAM") as dramp,
            tc.tile_pool(name="psA", bufs=2, space="PSUM") as psA,
            tc.tile_pool(name="psB", bufs=3, space="PSUM") as psB,
            tc.tile_pool(name="psC", bufs=2, space="PSUM") as psC,
            tc.tile_pool(name="psS", bufs=1, space="PSUM") as psS,
        ):
            IMW = [S0, S1]          # tokens per image
            IMO = [0, S0]           # global token offset per image
            NKV = [S0 // 128, S1 // 128]
            # ---- persistent tiles ----
            cos2 = constp.tile([128, S], bf16, tag="cos2")
            sin2 = constp.tile([128, S], bf16, tag="sin2")
            rotP = constp.tile([128, 128], bf16, tag="rotP")
            ones1 = constp.tile([128, 1], bf16, tag="ones1")
            onesr = constp.tile([1, 64], bf16, tag="onesr")
            onesrb = constp.tile([1, 128], bf16, tag="onesrb")
            epsc = constp.tile([128, 1], f32, tag="epsc")
            lnw = constp.tile([128, KT], f32, tag="lnw")
            nc.sync.dma_start(cos2[:], cos2_d[:])
            nc.sync.dma_start(sin2[:], sin2_d[:])
            nc.sync.dma_start(rotP[:], rotP_d[:])
            nc.sync.dma_start(lnw[:], lnw_d[:])
            nc.gpsimd.memset(ones1[:], 1.0)
            nc.gpsimd.memset(onesr[:], 1.0)
            nc.gpsimd.memset(onesrb[:], 1.0)
            nc.gpsimd.memset(epsc[:], EPS)

            # warmup collective: absorb initial core skew during conv
            warm = constp.tile([128, 8], f32, tag="warm")
            nc.gpsimd.memset(warm[:], 1.0)
            warm_i = dramp.tile([128, 8], f32, tag="warm_i")
            warm_o = dramp.tile([128, 8], f32, tag="warm_o",
                                addr_space="Shared")
            nc.gpsimd.dma_start(warm_i[:], warm[:])
            nc.gpsimd.collective_compute(
                "AllReduce", ALU.add, ins=[warm_i.opt()], outs=[warm_o.opt()],
                replica_groups=[list(range(NCORES))])

            def act_raw(out, in_, func, bias=0.0, scale=1.0):
                """activation() without the Rsqrt/Reciprocal accuracy guard."""
                eng = nc.scalar
                inputs = [eng.lower_ap(in_)]
                for arg in (bias, scale, 0.0):
                    if isinstance(arg, bass_mod.AP):
                        inputs.append(eng.lower_ap(arg))
                    else:
                        inputs.append(mybir.ImmediateValue(
                            dtype=f32, value=float(arg)))
                return eng.add_instruction(mybir.InstActivation(
                    name=f"I-{nc.next_id()}", func=func,
                    ins=inputs, outs=[eng.lower_ap(out)]))

            # per-image, per-ktile residual / normed tiles (fine-grain deps)
            resids = [[bigp.tile([128, IMW[i]], f32, tag=f"res{i}_{k}",
                                 name=f"res{i}_{k}") for k in range(KT)]
                      for i in range(2)]
            xnorms = [[bigp.tile([128, IMW[i]], bf16, tag=f"xn{i}_{k}",
                                 name=f"xn{i}_{k}") for k in range(KT)]
                      for i in range(2)]
            hmlps = [[bigp.tile([128, IMW[i]], bf16, tag=f"hm{i}_{m}",
                                name=f"hm{i}_{m}") for m in range(MT_I)]
                     for i in range(2)]
            # persistent V tiles: [kv-token(part), kvblk, 2*(64+1)] with the
            # ones (denominator) columns written once
            v2s = [bigp.tile([128, NKV[i], 130], bf16, tag=f"v2_{i}",
                             name=f"v2_{i}") for i in range(2)]
            for i in range(2):
                for kv in range(NKV[i]):
                    nc.gpsimd.memset(v2s[i][:, kv, 64:65], 1.0)
                    nc.gpsimd.memset(v2s[i][:, kv, 129:130], 1.0)

            # round-robin psum->sbuf evacuation across engines
            _rr = [0]

            def evac(dst, src):
                e = _rr[0] = (_rr[0] + 1) % 2
                if e == 0:
                    nc.scalar.activation(dst, src, AF.Copy)
                    return nc.scalar
                nc.vector.tensor_copy(dst, src)
                return nc.vector

            def rms_chunk(img, co, rstd0, wcol=None,
                          write_back_f32=False):
                csl = slice(co, co + CH)
                pss = psS.tile([1, CH], f32, tag="pss")
                for kt in range(KT):
                    sq = castp.tile([128, CH], bf16, tag="sq", bufs=4)
                    if kt % 2 == 0:
                        nc.vector.tensor_mul(sq[:], resids[img][kt][:, csl],
                                             resids[img][kt][:, csl])
                    else:
                        nc.scalar.activation(sq[:], resids[img][kt][:, csl],
                                             AF.Square)
                    nc.tensor.matmul(pss[:], lhsT=ones1[:], rhs=sq[:],
                                     start=(kt == 0), stop=(kt == KT - 1))
                act_raw(rstd0[:, csl], pss[:], AF.Rsqrt,
                        bias=epsc[0:1, :], scale=1.0 / HID)
                rstdb = psB.tile([128, 512], f32, tag="psb", name="rstdb")
                nc.tensor.matmul(rstdb[:, 0:CH], lhsT=onesrb[:],
                                 rhs=rstd0[0:1, csl], start=True, stop=True)
                rstdb = rstdb[:, 0:CH]
                csl2 = slice(0, CH)
                for kt in range(KT):
                    if wcol is None:
                        nc.vector.tensor_mul(xnorms[img][kt][:, csl],
                                             resids[img][kt][:, csl],
                                             rstdb[:, csl2])
                    else:
                        nc.vector.scalar_tensor_tensor(
                            xnorms[img][kt][:, csl],
                            resids[img][kt][:, csl],
                            wcol[:, kt:kt + 1], rstdb[:, csl2],
                            ALU.mult, ALU.mult)
                    if write_back_f32:
                        nc.scalar.activation(resids[img][kt][:, csl],
                                             xnorms[img][kt][:, csl],
                                             AF.Copy)

            def rms_norm(img, wcol=None, write_back_f32=False):
                W = IMW[img]
                rstd0 = wrk1p.tile([1, W], bf16, tag=f"rstd0{img}")
                for co in range(0, W, CH):
                    rms_chunk(img, co, rstd0, wcol, write_back_f32)

            # ---- conv patch embed (replicated, streamed) + ln_pre ----
            with tc.tile_pool(name="convp", bufs=2) as convp:
                for chi in range(NCH):
                    gco = chi * CH
                    img = 0 if gco < S0 else 1
                    lco = gco - IMO[img]
                    pch = convp.tile([128, PKT, CH], bf16, tag="pch")
                    nc.sync.dma_start(pch[:], patchesT_d[:, :, gco:gco + CH])
                    for kt in range(KT):
                        cwt = convp.tile([128, PKT, 128], bf16, tag="cwt")
                        nc.sync.dma_start(
                            cwt[:], convWT_d[:, :, kt * 128:(kt + 1) * 128])
                        psx = psA.tile([128, CH], f32, tag="psx")
                        for pk in range(PKT):
                            nc.tensor.matmul(
                                psx[:], lhsT=cwt[:, pk, :],
                                rhs=pch[:, pk, :],
                                start=(pk == 0), stop=(pk == PKT - 1))
                        nc.scalar.activation(
                            resids[img][kt][:, lco:lco + CH], psx[:],
                            AF.Copy)
            rms_norm(0, lnw, write_back_f32=True)
            rms_norm(1, lnw, write_back_f32=True)

            def qkv_attn(img, wq, wk, wv, wo):
                """Full attention for one image -> AR output dram tile."""
                W = IMW[img]
                lo = IMO[img]
                nq = W // CH
                nkv = NKV[img]
                v2 = v2s[img]
                qt = attp.tile([128, W], bf16, tag=f"qt{img}")
                kt_t = attp.tile([128, W], bf16, tag=f"kt{img}")
                # q/k projections (all chunks first: evacuations overlap)
                for dst, w in ((qt, wq), (kt_t, wk)):
                    for co in range(0, W, CH):
                        psq = psA.tile([128, CH], f32, tag="psx")
                        for kt in range(KT):
                            nc.tensor.matmul(
                                psq[:], lhsT=w[:, kt, :],
                                rhs=xnorms[img][kt][:, co:co + CH],
                                start=(kt == 0), stop=(kt == KT - 1))
                        nc.scalar.activation(dst[:, co:co + CH], psq[:],
                                             AF.Copy)
                # rope: rot = P @ x (PE), x = x*cos + rot*sin (vector)
                for dst in (qt, kt_t):
                    for co in range(0, W, CH):
                        csl = slice(co, co + CH)
                        gsl = slice(lo + co, lo + co + CH)
                        psr = psB.tile([128, 512], f32, tag="psb")
                        nc.tensor.matmul(psr[:, 0:CH], lhsT=rotP[:],
                                         rhs=dst[:, csl],
                                         start=True, stop=True)
                        t1 = castp.tile([128, CH], bf16, tag="t1", bufs=2)
                        t2 = castp.tile([128, CH], bf16, tag="t2", bufs=2)
                        nc.vector.tensor_mul(t1[:], dst[:, csl],
                                             cos2[:, gsl])
                        nc.vector.tensor_mul(t2[:], psr[:, 0:CH],
                                             sin2[:, gsl])
                        nc.vector.tensor_add(dst[:, csl], t1[:], t2[:])
                # v projection ([kv-token, 128ch] -> persistent v2 cols)
                for kv in range(nkv):
                    psv = psB.tile([128, 512], f32, tag="psb")
                    for kt in range(KT):
                        nc.tensor.matmul(
                            psv[:, 0:128],
                            lhsT=xnorms[img][kt][:, kv * 128:(kv + 1) * 128],
                            rhs=wv[:, kt, :],
                            start=(kt == 0), stop=(kt == KT - 1))
                    nc.vector.tensor_copy(v2[:, kv, 0:64], psv[:, 0:64])
                    nc.scalar.activation(v2[:, kv, 65:129], psv[:, 64:128],
                                         AF.Copy)
                # attention per q-chunk; kv-block pipelined 1 deep so the
                # exp of block i hides under the scores matmuls of i+1.
                # o-proj of chunk ci-1 is injected into chunk ci's stream so
                # the AllReduce input lands as early as possible.
                otcs = [attp.tile([128, CH], bf16, tag=f"otc{img}_{ci}",
                                  name=f"otc{img}_{ci}") for ci in range(nq)]
                arouts = []

                def oproj(ci):
                    arin = dramp.tile([128, KT, CH], bf16,
                                      tag=f"cai{img}{ci}",
                                      name=f"cai{img}{ci}")
                    aro = dramp.tile([128, KT, CH], bf16,
                                     tag=f"cao{img}{ci}",
                                     name=f"cao{img}{ci}",
                                     addr_space="Shared")
                    stage = stgp.tile([128, KT, CH], bf16, tag="stg")
                    for kt in range(KT):
                        pso = psA.tile([128, CH], f32, tag="psx")
                        nc.tensor.matmul(pso[:], lhsT=wo[:, kt, :],
                                         rhs=otcs[ci][:],
                                         start=True, stop=True)
                        evac(stage[:, kt, :], pso[:])
                        nc.scalar.dma_start(arin[:, kt, :], stage[:, kt, :])
                    nc.gpsimd.collective_compute(
                        "AllReduce", ALU.add,
                        ins=[arin.opt()], outs=[aro.opt()],
                        replica_groups=[list(range(NCORES))])
                    arouts.append((aro, ci * CH))

                for ci in range(nq):
                    qsl = slice(ci * CH, (ci + 1) * CH)
                    psavs = [psC.tile([65, CH], f32, tag="psav",
                                      name=f"psav{h}") for h in range(2)]
                    pts = {}

                    def issue_scores(i):
                        for h in range(2):
                            hsl = slice(h * 64, (h + 1) * 64)
                            pss = psB.tile([128, 512], f32, tag="psb")
                            nc.tensor.matmul(
                                pss[:, 0:CH],
                                lhsT=kt_t[hsl, i * 128:(i + 1) * 128],
                                rhs=qt[hsl, qsl], start=True, stop=True)
                            pt = castp.tile([128, CH], bf16, tag="pt",
                                            bufs=4, name=f"pt{h}")
                            nc.scalar.activation(pt[:], pss[:, 0:CH],
                                                 AF.Exp, scale=SCALE)
                            pts[(h, i)] = pt

                    issue_scores(0)
                    for i in range(nkv):
                        if i + 1 < nkv:
                            issue_scores(i + 1)
                        for h in range(2):
                            nc.tensor.matmul(
                                psavs[h][:],
                                lhsT=v2[:, i, h * 65:h * 65 + 65],
                                rhs=pts[(h, i)][:],
                                start=(i == 0), stop=(i == nkv - 1))
                    # denominators: reciprocal row -> K=1 matmul broadcast
                    psbc = psB.tile([128, 512], f32, tag="psb")
                    for h in range(2):
                        rec = castp.tile([1, CH], bf16, tag="rec", bufs=2)
                        act_raw(rec[:], psavs[h][64:65, :], AF.Reciprocal)
                        nc.tensor.matmul(
                            psbc[h * 64:(h + 1) * 64, 0:CH],
                            lhsT=onesr[:], rhs=rec[:],
                            start=True, stop=True, skip_group_check=True)
                    obc = castp.tile([128, CH], bf16, tag="obc", bufs=2)
                    nc.vector.tensor_copy(obc[:], psbc[:, 0:CH])
                    for h in range(2):
                        nc.vector.tensor_mul(
                            otcs[ci][h * 64:(h + 1) * 64, :],
                            psavs[h][0:64, :], obc[h * 64:(h + 1) * 64, :])
                for ci in range(nq):
                    oproj(ci)
                return arouts

            def add_ar(img, arouts, rms=False, write_out=False):
                lo = IMO[img]
                W = IMW[img]
                rstd0 = None
                if rms:
                    rstd0 = wrk1p.tile([1, W], bf16, tag=f"rstd0{img}")
                for aro, co in arouts:
                    csl = slice(co, co + CH)
                    for kt in range(KT):
                        arr = wrk2p.tile([128, CH], bf16, tag="arrc", bufs=4)
                        nc.sync.dma_start(arr[:], aro[:, kt, :])
                        nc.vector.tensor_add(resids[img][kt][:, csl],
                                             resids[img][kt][:, csl], arr[:])
                        if write_out:
                            nc.sync.dma_start(
                                out_d[:, kt, lo + co:lo + co + CH],
                                resids[img][kt][:, csl])
                    if rms:
                        rms_chunk(img, co, rstd0)

            def mlp(img, wg, wu, wd):
                W = IMW[img]
                arouts = []
                for co in range(0, W, CH):
                    ci = co // CH
                    csl = slice(co, co + CH)
                    for mt in range(MT_I):
                        msl = slice(mt * 128, (mt + 1) * 128)
                        psg = psA.tile([128, CH], f32, tag="psx")
                        for kt in range(KT):
                            nc.tensor.matmul(
                                psg[:], lhsT=wg[:, kt, msl],
                                rhs=xnorms[img][kt][:, csl],
                                start=(kt == 0), stop=(kt == KT - 1))
                        gts = castp.tile([128, CH], bf16, tag="gts")
                        nc.scalar.activation(gts[:], psg[:], AF.Silu)
                        psu = psB.tile([128, 512], f32, tag="psb")
                        for kt in range(KT):
                            nc.tensor.matmul(
                                psu[:, 0:CH], lhsT=wu[:, kt, msl],
                                rhs=xnorms[img][kt][:, csl],
                                start=(kt == 0), stop=(kt == KT - 1))
                        nc.vector.tensor_mul(hmlps[img][mt][:, csl], gts[:],
                                             psu[:, 0:CH])
                    arin = dramp.tile([128, KT, CH], bf16,
                                      tag=f"cmi{img}{ci}",
                                      name=f"cmi{img}{ci}")
                    aro = dramp.tile([128, KT, CH], bf16,
                                     tag=f"cmo{img}{ci}",
                                     name=f"cmo{img}{ci}",
                                     addr_space="Shared")
                    stage = stgp.tile([128, KT, CH], bf16, tag="stg")
                    for kt in range(KT):
                        psd = psA.tile([128, CH], f32, tag="psx")
                        for mt in range(MT_I):
                            nc.tensor.matmul(
                                psd[:],
                                lhsT=wd[:, mt, kt * 128:(kt + 1) * 128],
                                rhs=hmlps[img][mt][:, co:co + CH],
                                start=(mt == 0), stop=(mt == MT_I - 1))
                        evac(stage[:, kt, :], psd[:])
                        nc.scalar.dma_start(arin[:, kt, :], stage[:, kt, :])
                    nc.gpsimd.collective_compute(
                        "AllReduce", ALU.add,
                        ins=[arin.opt()], outs=[aro.opt()],
                        replica_groups=[list(range(NCORES))])
                    arouts.append((aro, co))
                return arouts

            # ---- transformer layers, software-pipelined across the MLP
            # AllReduce: layer l's MLP AR for image i is added at the top of
            # layer l+1 right before that image's attention norm. Image 1
            # (the small one) goes first so its AllReduce hides under image
            # 0's larger compute ----
            with tc.tile_pool(name="stg", bufs=2) as stgp:
                pend = [None, None]
                for l in range(NLAYERS):
                    wq = watp.tile([128, KT, 128], bf16, tag="wq")
                    wk = watp.tile([128, KT, 128], bf16, tag="wk")
                    wv = watp.tile([128, KT, 128], bf16, tag="wv")
                    wo = watp.tile([128, KT, 128], bf16, tag="wo")
                    nc.sync.dma_start(wq[:], wq_d[l])
                    nc.sync.dma_start(wk[:], wk_d[l])
                    nc.sync.dma_start(wv[:], wv_d[l])
                    nc.sync.dma_start(wo[:], wo_d[l])
                    wg = wmlpp.tile([128, KT, 512], bf16, tag="wg")
                    wu = wmlpp.tile([128, KT, 512], bf16, tag="wu")
                    wd = wdp.tile([128, MT_I, HID], bf16, tag="wd")
                    nc.sync.dma_start(wg[:], wg_d[l])
                    nc.sync.dma_start(wu[:], wu_d[l])
                    nc.sync.dma_start(wd[:], wd_d[l])

                    ar_a = [None, None]
                    for img in (1, 0):
                        if pend[img] is not None:
                            add_ar(img, pend[img], rms=True)
                            pend[img] = None
                        else:
                            rms_norm(img)
                        ar_a[img] = qkv_attn(img, wq, wk, wv, wo)
                    for img in (1, 0):
                        add_ar(img, ar_a[img], rms=True)
                        pend[img] = mlp(img, wg, wu, wd)

                for img in (1, 0):
                    add_ar(img, pend[img], write_out=True)

    nc.compile()
    return nc


# ---------------- host-side prep ----------------

def _im2col(img):
    C, H, W = img.shape
    h, w = H // PATCH, W // PATCH
    p = img.reshape(C, h, PATCH, w, PATCH).transpose(1, 3, 0, 2, 4)
    return p.reshape(h * w, C * PATCH * PATCH)


def _rope_tables():
    freqs = 1.0 / THETA ** (np.arange(0, HD, 2, dtype=np.float64) / HD)
    fh = np.outer(np.arange(MAXSIDE, dtype=np.float64), freqs[::2])
    fw = np.outer(np.arange(MAXSIDE, dtype=np.float64), freqs[1::2])
    pids = np.concatenate([
        (np.arange(h)[:, None] * MAXSIDE + np.arange(w)[None, :]).reshape(-1)
        for h, w in GRIDS])
    inv = np.concatenate([
        np.broadcast_to(fh[:, None, :], (MAXSIDE, MAXSIDE, HD // 4)),
        np.broadcast_to(fw[None, :, :], (MAXSIDE, MAXSIDE, HD // 4))],
        axis=-1).reshape(-1, HD // 2)
    inv = np.concatenate([inv, inv], axis=-1)
    emb = inv[pids]                                   # [S, 64]
    cosT = np.cos(emb).T.astype(np.float32)           # [64, S]
    sinT = np.sin(emb).T.astype(np.float32)
    sinTs = np.concatenate([-sinT[:32], sinT[32:]], axis=0)
    cos2 = np.concatenate([cosT, cosT], axis=0).astype(BF16)
    sin2 = np.concatenate([sinTs, sinTs], axis=0).astype(BF16)
    return np.ascontiguousarray(cos2), np.ascontiguousarray(sin2)


def _rot_perm():
    """rot[m] = q[perm(m)] permutation as a [k, m] matmul constant."""
    P = np.zeros((128, 128), np.float32)
    for b in (0, 64):
        for m in range(32):
            P[b + 32 + m, b + m] = 1.0          # rot[m] = q[m+32]
            P[b + m, b + 32 + m] = 1.0          # rot[m+32] = q[m]
    return P.astype(BF16)


def _ktile(a, last):
    """[L, 1024, last] -> [L, 128, kt, last] (partition-major k-tiles)."""
    L = a.shape[0]
    return np.ascontiguousarray(
        a.reshape(L, -1, 128, last).transpose(0, 2, 1, 3))


def _prep(inputs):
    f32 = np.float32
    patches = np.concatenate([
        _im2col(np.asarray(inputs["img0"], f32)),
        _im2col(np.asarray(inputs["img1"], f32))])          # [S, 768]
    patchesT = np.ascontiguousarray(
        patches.T.reshape(PKT, 128, S).transpose(1, 0, 2)).astype(BF16)
    cw = np.asarray(inputs["conv_w"], f32).reshape(HID, 768)
    convWT = np.ascontiguousarray(
        cw.T.reshape(PKT, 128, HID).transpose(1, 0, 2)).astype(BF16)
    cos2, sin2 = _rope_tables()
    lnw = np.ascontiguousarray(
        np.asarray(inputs["ln_pre_w"], f32).reshape(KT, 128).T)

    anw = np.asarray(inputs["attn_norm_w"], f32)[:, :, None]  # [4, in, 1]
    fnw = np.asarray(inputs["ffn_norm_w"], f32)[:, :, None]
    qwT = np.asarray(inputs["q_w"], f32).transpose(0, 2, 1) * anw
    kwT = np.asarray(inputs["k_w"], f32).transpose(0, 2, 1) * anw
    vwT = np.asarray(inputs["v_w"], f32).transpose(0, 2, 1) * anw
    owT = np.asarray(inputs["o_w"], f32).transpose(0, 2, 1)   # [4, d, e]
    gwT = np.asarray(inputs["gate_w"], f32).transpose(0, 2, 1) * fnw
    uwT = np.asarray(inputs["up_w"], f32).transpose(0, 2, 1) * fnw
    dwT = np.asarray(inputs["down_w"], f32).transpose(0, 2, 1)  # [4, I, out]

    common = dict(patchesT=patchesT, convWT=convWT, cos2=cos2, sin2=sin2,
                  rotP=_rot_perm(), lnw=lnw)
    in_maps = []
    for c in range(NCORES):
        esl = slice(c * 128, (c + 1) * 128)
        isl = slice(c * 512, (c + 1) * 512)
        wo = owT[:, esl, :]                                   # [4, 128, 1024]
        m = dict(
            wq=_ktile(qwT[:, :, esl].astype(BF16), 128),
            wk=_ktile(kwT[:, :, esl].astype(BF16), 128),
            wv=_ktile(vwT[:, :, esl].astype(BF16), 128),
            wo=np.ascontiguousarray(
                wo.reshape(NLAYERS, 128, KT, 128)).astype(BF16),
            wg=_ktile(gwT[:, :, isl].astype(BF16), 512),
            wu=_ktile(uwT[:, :, isl].astype(BF16), 512),
            wd=np.ascontiguousarray(
                dwT[:, isl, :].reshape(NLAYERS, MT_I, 128, HID)
                .transpose(0, 2, 1, 3)).astype(BF16),
            **common)
        in_maps.append(m)
    return in_maps


LAST_RESULTS = None
TRACE = False


def _install_ntff_hook():
    """The RL container's antenv lacks axon_hooks; recreate it so
    trace=True can capture NTFF profiles through the axon terminal."""
    import types
    import antenv

    if hasattr(antenv, "axon_hooks"):
        return
    mod = types.ModuleType("antenv.axon_hooks")
    holder = [None]
    mod.set_axon_ntff_profile_hook = lambda h: holder.__setitem__(0, h)
    mod.get_axon_ntff_profile_hook = lambda: holder[0]
    sys.modules["antenv.axon_hooks"] = mod
    antenv.axon_hooks = mod
    if "/root/.axon_site" not in sys.path:
        sys.path.insert(0, "/root/.axon_site")
    try:
        from trn_agent_boot.trn_boot import _ntff_profile_via_ctypes
        mod.set_axon_ntff_profile_hook(
            _ntff_profile_via_ctypes("/opt/axon/libaxon_pjrt.so"))
    except Exception as e:  # pragma: no cover
        print("ntff hook install failed:", e)


def kernel(**inputs):
    global LAST_RESULTS
    from concourse import bass_utils

    if TRACE:
        _install_ntff_hook()

    if "nc" not in _CACHE:
        _CACHE["nc"] = _build_nc()
    nc = _CACHE["nc"]
    in_maps = _prep(inputs)
    res = bass_utils.run_bass_kernel_spmd(
        nc, in_maps, core_ids=list(range(NCORES)), trace=TRACE)
    LAST_RESULTS = res
    out = res.results[0]["out"]                 # [128, KT, S] f32
    full = out.transpose(1, 0, 2).reshape(HID, S)   # [hid, S]
    return np.ascontiguousarray(full.T[None]).astype(np.float32)
